# revision 47
# baseline (speedup 1.0000x reference)
"""Trainium2 Bass kernel for a binarized (XNOR-style) ResNet BasicBlock.

Reference semantics (per nn_BasicBlock_37228776522124):
    out = BN2(conv3x3(sign(BN1(conv3x3(sign(x), sign(w1)*a1))), sign(w2)*a2)) + x
with training-mode BN (batch stats over N,H,W) and per-out-channel
weight scale a_l = mean(|w_l|).

Key facts exploited:
  * conv inputs are exactly +-1 -> fp8 DoubleRow matmuls accumulate EXACT
    integers in fp32 PSUM (|z| <= 2304 < 2^24).
  * conv(sign(x), sign(w)*a) = a * conv(sign(x), sign(w)); a and BN fold
    into one per-channel affine s*z + b applied post-conv.
  * z is always even; z1 only feeds sign(z - mean) so it is stored as fp8
    at z/16 (sign-safe); z2 is stored fp16 at z/2 (exact).
  * Data-parallel over batch (4 images/core on 8 cores). BN batch stats
    are AllReduced in TWO slices per conv: images 0-2 (hidden under
    image 3's conv, absorbs cross-core launch skew) and image 3 alone
    (short critical-path AR at pure collective latency).
  * x is kept resident in SBUF as fp16 for the residual; w arrives
    host-cast to bf16 (sign-exact, alpha error ~1e-4 rel); the output is
    written fp16 and converted to fp32 on the host (|err| ~1e-3 << tol).
  * The activation pad ring is memset once via narrow ring slices; weight
    transposes run at bf16 rate with 4 taps batched per PSUM tile and one
    batched Sign drain each.
  * Bulk DMAs ride only the sync+scalar HWDGE queues (gpsimd DMA is a
    slow software path reserved for the collectives' tiny transfers).
  * Junk matmul chains bridge PE idle windows (before conv1 and across
    the BN1 AllReduce) to keep the PE HAM clock-gate at 2.4 GHz.

Self-contained: only needs /opt/trn_rl_repo (the Bass toolchain) + numpy.
"""

import os
import sys

for _p in ("/opt/trn_rl_repo",):
    if os.path.isdir(_p) and _p not in sys.path:
        sys.path.insert(0, _p)

import numpy as np

# Problem shapes (hardcoded per spec)
N_FULL, C, H, W = 32, 256, 56, 56
NCORES = 8
NPER = N_FULL // NCORES          # 4 images per core
SP = H * W                       # 3136
HP = H + 2                       # 58 (zero-padded)
SPP = HP * HP                    # 3364
NIB = C // 128                   # 2 input-channel blocks
NOB = C // 128                   # 2 output-channel blocks
NTAP = 9
NK = NTAP * NIB                  # 18 accumulation steps per output tile
RB = 7                           # row-blocks of 8 rows
RBW = 8 * W                      # 448 valid outputs per row-block
NMOV = 8 * HP                    # 464 moving columns (8 contiguous pad rows)
RBQ = NMOV + 2                   # 466 f32 <= one psum bank; tap tw writes a
                                 # contiguous 464 window at offset 2-tw; all
                                 # taps agree on q = r*58 + w + 2 for valid w
EPS = 1e-5
KELEM = C * NTAP                 # 2304 weight elems per out channel
WS = NOB * NTAP * 128            # wsign cols per i-block (ib-major layout)

_nc_cache = {}


def build_nc(num_devices=NCORES):
    import concourse.bacc as bacc
    import concourse.tile as tile
    import concourse.mybir as mybir
    from concourse.masks import make_identity

    F32 = mybir.dt.float32
    F16 = mybir.dt.float16
    BF16 = mybir.dt.bfloat16
    ALU = mybir.AluOpType
    ACTF = mybir.ActivationFunctionType
    AX = mybir.AxisListType

    nc = bacc.Bacc(
        "TRN2", target_bir_lowering=False, debug=False,
        num_devices=num_devices,
    )

    # x arrives pre-cast to fp16 (host-side): sign(x) is exact in fp16 for
    # randn inputs (subnormals keep their sign) and fp16 carries 10 mantissa
    # bits for the residual add; w arrives pre-cast to bf16 and pre-flattened
    # [O, I*9] (sign-exact; alpha from |w_bf16| deviates ~1e-4 rel). Both
    # halve the startup DMA, which otherwise paces conv1's start.
    x_t = nc.dram_tensor("x", [NPER, C, H, W], F16, kind="ExternalInput")
    w_t = [
        nc.dram_tensor("w1b", [C, KELEM], BF16, kind="ExternalInput"),
        nc.dram_tensor("w2b", [C, KELEM], BF16, kind="ExternalInput"),
    ]
    g_t = [
        nc.dram_tensor("gamma1", [C], F32, kind="ExternalInput"),
        nc.dram_tensor("gamma2", [C], F32, kind="ExternalInput"),
    ]
    b_t = [
        nc.dram_tensor("beta1", [C], F32, kind="ExternalInput"),
        nc.dram_tensor("beta2", [C], F32, kind="ExternalInput"),
    ]
    # fp16 output: host converts back to fp32 (halves the tail DMA)
    out_t = nc.dram_tensor("out", [NPER, C, H, W], F16, kind="ExternalOutput")

    x_ap = x_t.ap().rearrange("n c h w -> n c (h w)")      # [4, 256, 3136]
    out_ap = out_t.ap().rearrange("n c h w -> n c (h w)")
    rgroups = [list(range(num_devices))]
    M_TOTAL = float(num_devices * NPER * SP)
    # debug bisection: W < C1 < AR1 < C2 < FULL
    phase_lim = {"W": 0, "C1": 1, "AR1": 2, "C2": 3, "FULL": 9}[
        os.environ.get("KERNEL_PHASES", "FULL")]
    no_mm = bool(os.environ.get("KERNEL_NO_MM"))
    # fp8e4m3 DoubleRow matmuls: +-1 exact in fp8, 2 K-rows/cell -> ~2x PE
    use_fp8 = os.environ.get("KERNEL_FP8", "1") == "1"
    A8 = mybir.dt.float8e4
    PM = mybir.MatmulPerfMode
    # abuf block pitch: 2-col left margin (first-tap 466-wide matmul reads
    # from grid-2) + 3364 grid + tail pad; 3376 keeps fp8 pair-step 16B-aligned
    ABW = 3376
    GB = 2                          # grid base offset inside each block
    ABD = A8 if use_fp8 else BF16
    NWARM1 = int(os.environ.get("KERNEL_NWARM1", "20"))
    NBRIDGE = int(os.environ.get("KERNEL_NBRIDGE", "24"))

    with tile.TileContext(nc) as tc:
        with (
            tc.tile_pool(name="consts", bufs=1) as pc,
            tc.tile_pool(name="dbl", bufs=2) as pd,
            tc.tile_pool(name="psum", bufs=8, space="PSUM") as pp,
            tc.tile_pool(name="dram", bufs=1, space="DRAM") as pdram,
        ):
            # bf16 identity: transposes stream it as the moving operand at
            # bf16 rate (one pass) instead of fp32 LOW/HIGH (two passes)
            identB = pc.tile([128, 128], BF16, name="identB", tag="identB")
            make_identity(nc, identB[:])
            epsap = pc.tile([128, 1], F32, name="epsap", tag="epsap")
            nc.vector.memset(epsap[:], EPS)
            # junk fp8 operands for PE HAM warm-up matmul chains
            jw = pc.tile([128, 512], A8, name="jw", tag="jw")
            nc.vector.memset(jw[:], 1.0)

            # persistent stores: z1 fp8 at z/16 (feeds sign only), z2 fp16
            # at z/2 (exact, feeds the output affine)
            zstore = [
                pc.tile([128, NPER * NOB * SP], A8 if l == 0 else F16,
                        name=f"z{l}", tag=f"z{l}")
                for l in range(2)
            ]
            zscale = [1.0 / 16.0, 0.5]
            # x resident in SBUF (fp16, DMA'd straight from DRAM): feeds
            # both the sign fills and the finalize residual add. One tile
            # per image -- a single tile would add a false tile-level WAR
            # from each image's DMA onto the previous image's sign reads.
            x16n = [pc.tile([128, NOB * SP], F16, name=f"x16{n}",
                            tag=f"x16{n}") for n in range(NPER)]
            # tiny marker tile: re-reads a few elements of w1b so the PE
            # warm-up chain can gate on the critical loads' arrival
            jx = pc.tile([128, 4], BF16, name="jx", tag="jx")
            jw1b = pc.tile([128, 4], A8, name="jw1b", tag="jw1b")
            # two persistent activation buffers, alternated per image; the
            # zero pad ring + margins are written ONCE via narrow ring
            # memsets (the interior is fully overwritten by each image's
            # sign fill) -- full-buffer memsets would cost 2x5.7us on DVE.
            ab2 = [pc.tile([128, NIB * ABW], ABD, name=f"ab{i}", tag=f"ab{i}")
                   for i in range(2)]
            with tc.high_priority():
                for i in range(2):
                    for ib in range(NIB):
                        blk = ab2[i][:, ib * ABW:(ib + 1) * ABW]
                        # margin(2) + pad row 0 (58) + row1 col0 -> cols 0..60
                        nc.vector.memset(blk[:, 0:61], 0.0)
                        # row56 col57 + pad row 57 + tail pad -> 3307..3376
                        nc.vector.memset(blk[:, 3307:ABW], 0.0)
                        # rows 1..55: (row r col57, row r+1 col0) pairs,
                        # start col 2+58*1+57 = 117, stride 58
                        ring = blk[:, 117:117 + 55 * 58].rearrange(
                            "p (r c) -> p r c", c=58)[:, :, 0:2]
                        nc.vector.memset(ring, 0.0)
            # wsign layout: [ib][ob][tap] blocks of 128 cols; the DoubleRow
            # lhsT pair-step is WS=2304 fp8 bytes (16B-aligned) and same-ib
            # taps are contiguous so 4-tap Sign drains batch into one op.
            wsign = [
                pc.tile([128, NIB * WS], ABD, name=f"ws{l}", tag=f"ws{l}")
                for l in range(2)
            ]
            alphar = [pc.tile([128, NOB], F32, name=f"al{l}", tag=f"al{l}") for l in range(2)]
            sumc = [pc.tile([128, NOB * 28], F32, name=f"sc{l}", tag=f"sc{l}") for l in range(2)]
            sqc = [pc.tile([128, NOB * 28], F32, name=f"qc{l}", tag=f"qc{l}") for l in range(2)]
            # single stats AR per conv: the collective completes at
            # (slowest core's image-3 drain) + pure latency; a split would
            # only serialize behind the earlier AR on the in-order CC queue.
            NPART = 1
            PLIM = [(0, 28)]
            statp = [[pc.tile([128, 4], F32, name=f"sl{l}{p}", tag=f"sl{l}{p}")
                      for p in range(NPART)] for l in range(2)]
            statg2 = [[pc.tile([128, 4], F32, name=f"sg{l}{p}",
                               tag=f"sg{l}{p}")
                       for p in range(NPART)] for l in range(2)]
            # conv2 ib1-plane correction: its activations are encoded {0,2}
            # = sign+1 (one-op DVE fill), so z2_raw = z2 + corr with
            # corr[o] = sum of ib1-taps' signed weights; stats and coefs are
            # fixed up algebraically in fold2 (z2 is exact fp16, no
            # precision risk). cols: 0:2 c, 2:4 c/2, 4:6 4c, 6:8 c^2,
            # 8:10 scratch.
            corrt = pc.tile([128, 10], F32, name="corrt", tag="corrt")
            gb = [pc.tile([128, 2 * NOB], F32, name=f"gb{l}", tag=f"gb{l}") for l in range(2)]
            coef = [pc.tile([128, 2 * NOB], F32, name=f"cf{l}", tag=f"cf{l}") for l in range(2)]
            btmp = [pc.tile([128, 16], F32, name=f"bt{l}", tag=f"bt{l}") for l in range(2)]

            # ---------------- head-of-queue DMA triggers ----------------
            # sync: x-ib0 image 0, then x images 1-3 (SP engine: triggers
            #   cost no ACT time).
            # scalar: x-ib1 image 0, w1b obs, gamma/beta, w2b obs.
            wraw = [[pc.tile([128, KELEM], BF16, name="wraw", tag="wraw",
                             bufs=2) for _ in range(NOB)] for l in range(2)]
            SPH = SP // 2
            nc.sync.dma_start(x16n[0][:, 0:SP], x_ap[0, 0:128, :])
            nc.scalar.dma_start(x16n[0][:, SP:SP + SPH],
                                x_ap[0, 128:256, 0:SPH])
            nc.scalar.dma_start(wraw[0][0][:], w_t[0].ap()[0:128, :])
            nc.scalar.dma_start(x16n[0][:, SP + SPH:2 * SP],
                                x_ap[0, 128:256, SPH:SP])
            nc.scalar.dma_start(jx[:], w_t[0].ap()[0:128, 0:4])
            nc.scalar.dma_start(wraw[0][1][:], w_t[0].ap()[128:256, :])

            # dummy AllReduce at kernel start: absorbs the first-collective
            # latency concurrently with conv1 so the real ARs are fast
            ard_i = pdram.tile([128, 1], F32, name="ard_i", tag="ard_i")
            ard_o = pdram.tile([128, 1], F32, name="ard_o", tag="ard_o")
            nc.gpsimd.dma_start(ard_i[:], g_t[0].ap()[0:128])
            nc.gpsimd.collective_compute(
                "AllReduce", ALU.add, replica_groups=rgroups,
                ins=[ard_i.opt()], outs=[ard_o.opt()],
            )
            # (the DCE-keeping park of ard_o is emitted after conv1 so its
            # wait doesn't head-block a queue anything rides on)

            # ---------------- weight prep ----------------
            def trigger_w2_loads():
                # emitted from conv1's image-2 iteration: the scalar-queue
                # trigger instruction issues mid-conv1 (not behind all of
                # conv1's ACT compute) and the wraw buffer WAR from conv1's
                # weight prep is already satisfied, so w2 lands well before
                # its transposes in the AR1 window.
                for ob in range(NOB):
                    nc.scalar.dma_start(
                        wraw[1][ob][:],
                        w_t[1].ap()[ob * 128:(ob + 1) * 128, :])

            def weight_prep_ob(l, ob, demote):
                wr = wraw[l][ob]
                # alpha_raw = sum |w|, off-critical (fold-time only)
                with tc.high_priority(offset=-6000):
                    nc.vector.tensor_reduce(
                        out=alphar[l][:, ob:ob + 1], in_=wr[:],
                        axis=AX.X, op=ALU.add,
                        apply_absolute_value=True,
                    )
                # tap-major transposes, 4 taps batched per PSUM tile
                # with one batched Sign drain each (ACT reads 512 cols
                # of PSUM in one op instead of 4x128).
                wtap = wr[:].rearrange("p (i t) -> p t i", t=NTAP)
                with tc.high_priority(offset=demote):
                    for ib in range(NIB):
                        for t0 in range(0, NTAP, 4):
                            nt = min(4, NTAP - t0)
                            psT = pp.tile([128, 512], BF16,
                                          name="cps", tag="cps")
                            for j in range(nt):
                                nc.tensor.transpose(
                                    psT[:, j * 128:(j + 1) * 128],
                                    wtap[:, t0 + j, ib * 128:(ib + 1) * 128],
                                    identB[:],
                                )
                            nc.scalar.activation(
                                out=wsign[l][:,
                                             ib * WS + (ob * NTAP + t0) * 128:
                                             ib * WS + (ob * NTAP + t0 + nt) * 128],
                                in_=psT[:, 0:nt * 128], func=ACTF.Sign,
                            )

            def weight_prep_tail(l):
                # gamma/beta -> [128, col]
                for ob in range(NOB):
                    nc.scalar.dma_start(
                        gb[l][:, ob:ob + 1],
                        g_t[l].ap()[ob * 128:(ob + 1) * 128],
                    )
                    nc.scalar.dma_start(
                        gb[l][:, NOB + ob:NOB + ob + 1],
                        b_t[l].ap()[ob * 128:(ob + 1) * 128],
                    )
                # off-critical fold precomputes: alp = alpha_raw/KELEM,
                # aa = alp^2, agz = gamma*alp/zscale
                pre = btmp[l]
                with tc.high_priority(offset=-6000):
                    nc.vector.tensor_scalar_mul(
                        pre[:, 6:8], alphar[l][:, 0:2], 1.0 / KELEM)
                    nc.vector.tensor_mul(pre[:, 0:2], pre[:, 6:8], pre[:, 6:8])
                    nc.vector.tensor_mul(pre[:, 2:4], pre[:, 6:8], gb[l][:, 0:2])
                    nc.vector.tensor_scalar_mul(
                        pre[:, 2:4], pre[:, 2:4], 1.0 / zscale[l])

            def weight_prep(l):
                if l == 0:
                    # ob1's prep is NOT emitted here: the compile-time
                    # scheduler's DMA model is optimistic and would place
                    # its transposes ahead of image 0's matmuls in the PE
                    # queue, stalling conv1 ~11us until w1-ob1 really
                    # lands. It is emitted from conv_pass between image
                    # 0's ob0 and ob1 matmul blocks instead (program order
                    # pins the queue order).
                    weight_prep_ob(0, 0, None)
                else:
                    weight_prep_ob(1, 0, -150)
                    weight_prep_ob(1, 1, -150)
                    weight_prep_tail(1)

            def lhsT_for(l, ob, t):
                base = (ob * NTAP + t) * 128
                return wsign[l][:].rearrange(
                    "p (i x) -> p i x", i=NIB)[:, :, base:base + 128]

            weight_prep(0)

            # ---------------- stats AllReduce (split) ----------------
            arin = [[pdram.tile([128, 4], F32, name=f"ari{l}{p}", tag=f"ari{l}{p}")
                     for p in range(NPART)] for l in range(2)]
            arout = [[pdram.tile([128, 4], F32,
                                 name=f"aro{l}{p}", tag=f"aro{l}{p}")
                      for p in range(NPART)] for l in range(2)]

            def issue_stats_ar(l, part):
                lo, hi = PLIM[part]
                st = statp[l][part]
                with tc.high_priority(offset=None):
                    for ob in range(NOB):
                        nc.vector.tensor_reduce(
                            out=st[:, ob:ob + 1],
                            in_=sumc[l][:, ob * 28 + lo:ob * 28 + hi],
                            axis=AX.X, op=ALU.add,
                        )
                        nc.vector.tensor_reduce(
                            out=st[:, NOB + ob:NOB + ob + 1],
                            in_=sqc[l][:, ob * 28 + lo:ob * 28 + hi],
                            axis=AX.X, op=ALU.add,
                        )
                # arin DMAs ride the sync HWDGE queue (they only wait on
                # local stats, so they never head-block each other); the
                # DMA-backs ride the gpsimd queue, which collective_compute
                # already blocks until the AR completes -- putting them
                # anywhere else would head-block that queue on the AR.
                nc.sync.dma_start(arin[l][part][:], st[:])
                nc.gpsimd.collective_compute(
                    "AllReduce", ALU.add, replica_groups=rgroups,
                    ins=[arin[l][part].opt()], outs=[arout[l][part].opt()],
                )
                nc.gpsimd.dma_start(statg2[l][part][:], arout[l][part][:])

            def fold_bn(l):
                # with sm = Ssum/M (z'-units, sm = zs*mean) and e2 = Ssq/M:
                #   var = e2 - sm^2/zs^2 ; inv = rsqrt(aa*var + eps)
                #   coef_s = agz*inv ; coef_b = beta - coef_s*sm
                # (aa, agz precomputed off-critical in weight_prep)
                # l==1: the raw stats include the {0,2}-encoding correction
                # c: sm_t = sm_raw - c/2, e2_t = e2_raw - 4c*sm_raw + c^2.
                # coef_b conveniently uses sm_RAW: beta - s*(sm_t + c/2).
                pre = btmp[l]
                nc.vector.tensor_scalar_mul(
                    pre[:, 4:8], statg2[l][0][:, 0:4], 1.0 / M_TOTAL)
                if l == 1:
                    sm, e2 = corrt[:, 8:10], btmp[1][:, 14:16]
                    nc.vector.tensor_mul(e2, pre[:, 4:6], corrt[:, 4:6])
                    nc.vector.tensor_sub(e2, pre[:, 6:8], e2)
                    nc.vector.tensor_add(e2, e2, corrt[:, 6:8])
                    nc.vector.tensor_sub(sm, pre[:, 4:6], corrt[:, 2:4])
                else:
                    sm, e2 = pre[:, 4:6], pre[:, 6:8]
                w = pre[:, 8:10]
                nc.vector.tensor_mul(w, sm, sm)
                nc.vector.tensor_scalar_mul(
                    w, w, 1.0 / (zscale[l] * zscale[l]))
                nc.vector.tensor_sub(w, e2, w)                    # var
                nc.vector.tensor_mul(w, w, pre[:, 0:2])           # aa*var
                nc.scalar.activation(pre[:, 10:12], w, ACTF.Sqrt, bias=epsap[:])
                nc.vector.reciprocal(w, pre[:, 10:12])            # inv
                nc.vector.tensor_mul(coef[l][:, 0:2], pre[:, 2:4], w)
                nc.vector.tensor_mul(w, coef[l][:, 0:2], pre[:, 4:6])
                nc.vector.tensor_sub(coef[l][:, 2:4], gb[l][:, 2:4], w)
                if l == 0:
                    # thr = -coef_b/coef_s for fill2's DVE-emulated sign
                    # (gamma > 0 so coef_s > 0 and sign(s*z+b)=sign(z-thr))
                    thr = btmp[0][:, 14:16]
                    nc.vector.reciprocal(thr, coef[0][:, 0:2])
                    nc.vector.tensor_mul(thr, thr, coef[0][:, 2:4])
                    nc.vector.tensor_scalar_mul(thr, thr, -1.0)

            # ---------------- one conv pass (shared for conv1/conv2) --------
            def conv_pass(l, act_fill, do_ar=True):
                """act_fill(n, abuf) writes signed fp8 acts into the padded
                [128, NIB*SPP] buffer interior (ring already zero).
                Image n+1's fill is emitted BEFORE image n's matmuls/drains
                so on the in-order ACT/DVE/sync queues the next image's
                loads and signs run during the current matmul stream."""
                act_fill(0, ab2[0])
                if l == 0:
                    # PE HAM warm-up gated on the critical loads' arrival
                    # (jw1b is written from the jx marker): the junk-matmul
                    # chain warms the clock-gate during the fills.
                    with tc.high_priority():
                        nc.vector.tensor_scalar(
                            out=jw1b[:], in0=jx[:],
                            scalar1=0.0, scalar2=None, op0=ALU.mult)
                    warmps = pp.tile([128, 512], F32, name="cps", tag="cps")
                    for i in range(NWARM1):
                        nc.tensor.matmul(warmps[:, 0:128], jw[:, 0:128],
                                         jw[:, 0:128],
                                         start=(i == 0),
                                         stop=(i == NWARM1 - 1))
                        if i == 0:
                            nc.tensor.matmul(warmps[:, 0:4], jw[:, 0:128],
                                             jw1b[:],
                                             start=False, stop=False)
                    nc.vector.tensor_reduce(
                        out=btmp[0][:, 13:14], in_=warmps[:, 0:128],
                        axis=AX.X, op=ALU.add)
                for n in range(NPER):
                    abuf = ab2[n % 2]
                    if n + 1 < NPER:
                        act_fill(n + 1, ab2[(n + 1) % 2])
                    if l == 0 and n == 2:
                        trigger_w2_loads()
                    for ob in range(NOB):
                        if l == 0 and n == 0 and ob == 1:
                            # w1-ob1 prep lands here in program order: after
                            # image 0's ob0 matmuls, right before its first
                            # consumer (image 0's ob1 stream)
                            weight_prep_ob(0, 1, 0)
                            weight_prep_tail(0)
                        ps = [pp.tile([128, RBQ], F32, name="cps", tag="cps")
                              for _ in range(RB)]
                        if use_fp8:
                            ab3 = abuf[:].rearrange(
                                "p (two s) -> p two s", two=NIB)
                            for t in range(NTAP):
                                th, tw = t // 3, t % 3
                                lhsT = lhsT_for(l, ob, t)
                                for rb in range(RB):
                                    r0 = (rb * 8 + th) * HP
                                    if t == 0:
                                        # 466-wide: covers the whole psum
                                        # tile so has_written is uniform
                                        rhs = ab3[:, :, r0:r0 + RBQ]
                                        outap = ps[rb][:, 0:RBQ]
                                    else:
                                        rhs = ab3[:, :, GB + r0:GB + r0 + NMOV]
                                        outap = ps[rb][:, 2 - tw:2 - tw + NMOV]
                                    nc.tensor.matmul(
                                        outap, lhsT, rhs,
                                        start=(t == 0), stop=(t == NTAP - 1),
                                        perf_mode=PM.DoubleRow,
                                    )
                        else:
                            for k in range(NK):
                                t, ib = k // NIB, k % NIB
                                th, tw = t // 3, t % 3
                                af = abuf[:, ib * ABW:(ib + 1) * ABW]
                                lhsT = wsign[l][:, ib * WS + (ob * NTAP + t) * 128:
                                               ib * WS + (ob * NTAP + t + 1) * 128]
                                for rb in range(RB):
                                    r0 = (rb * 8 + th) * HP
                                    if no_mm and k > 0:
                                        continue
                                    if k == 0:
                                        rhs = af[:, r0:r0 + RBQ]
                                        outap = ps[rb][:, 0:RBQ]
                                    else:
                                        rhs = af[:, GB + r0:GB + r0 + NMOV]
                                        outap = ps[rb][:, 2 - tw:2 - tw + NMOV]
                                    nc.tensor.matmul(
                                        outap, lhsT, rhs,
                                        start=(k == 0),
                                        stop=(k == NK - 1) or no_mm,
                                    )
                        zs = zstore[l]
                        for rb in range(RB):
                            col = n * RB + rb
                            zsl = zs[:, ((n * NOB + ob) * SP + rb * RBW):
                                      ((n * NOB + ob) * SP + (rb + 1) * RBW)
                                      ].rearrange("p (h w) -> p h w", w=W)
                            qv = ps[rb][:, 2:2 + NMOV].rearrange(
                                "p (h w) -> p h w", w=HP)[:, :, 0:W]
                            # z*zscale store on DVE; accum_out = sum(z*zscale)
                            nc.vector.tensor_scalar(
                                out=zsl, in0=qv,
                                scalar1=zscale[l], scalar2=None, op0=ALU.mult,
                                op1=ALU.add,
                                accum_out=sumc[l][:, ob * 28 + col:
                                                  ob * 28 + col + 1],
                            )
                            # scr = z^2 (dummy out); accum = sum(z^2)
                            scr = pd.tile([128, RBW], F16, name="scr", tag="scr")
                            nc.scalar.activation(
                                out=scr[:].rearrange("p (h w) -> p h w", w=W),
                                in_=qv, func=ACTF.Square,
                                accum_out=sqc[l][:, ob * 28 + col:
                                                 ob * 28 + col + 1],
                            )
                if not do_ar:
                    return
                issue_stats_ar(l, 0)
                fold_bn(l)

            # ---------------- conv1: acts = sign(x) ----------------
            # x is DMA'd straight into the resident fp16 x16 tiles; image 0
            # splits its two plane-signs over ACT (ib0) and DVE (ib1,
            # emulated sign) so the fill latency before the first matmul is
            # halved. Images 1-3 load both planes on the sync queue (SP
            # triggers cost no ACT time) and sign on ACT in conv slack.
            def fill1(n, abuf):
                if n > 0:
                    for ib in range(NIB):
                        nc.sync.dma_start(
                            x16n[n][:, ib * SP:(ib + 1) * SP],
                            x_ap[n, ib * 128:(ib + 1) * 128, :])
                for ib in range(NIB):
                    a58 = abuf[:, ib * ABW + GB:ib * ABW + GB + SPP
                               ].rearrange("p (h w) -> p h w", w=HP)
                    av = a58[:, 1:H + 1, 1:W + 1]
                    xv = x16n[n][:, ib * SP:(ib + 1) * SP].rearrange(
                        "p (h w) -> p h w", w=W)
                    prio = None if n == 0 else (0 if n == 1 else -100)
                    with tc.high_priority(offset=prio):
                        if n == 0 and ib == 1:
                            # DVE-emulated sign (x >= 0)*2 - 1, in h-halves
                            # tracking the interleaved x-ib1 chunk DMAs
                            for h in range(2):
                                avh = a58[:, 1 + h * 28:1 + (h + 1) * 28,
                                          1:W + 1]
                                xvh = xv[:, h * 28:(h + 1) * 28, :]
                                nc.vector.tensor_scalar(
                                    out=avh, in0=xvh,
                                    scalar1=0.0, scalar2=2.0,
                                    op0=ALU.is_ge, op1=ALU.mult)
                                nc.vector.tensor_scalar_add(avh, avh, -1.0)
                        else:
                            nc.scalar.activation(
                                out=av, in_=xv, func=ACTF.Sign)

            if phase_lim >= 1:
                conv_pass(0, fill1, do_ar=(phase_lim >= 2))

            # park the dummy-AR result so DCE keeps it; the gpsimd queue
            # already waits for the collective, so the park adds no blocking
            # there (on any other queue the scheduler may slot it mid-stream
            # and stall that queue until the dummy AR completes).
            nc.gpsimd.dma_start(btmp[0][:, 12:13], ard_o[:])

            # conv2 weight prep here: its PE transposes run right after
            # conv1's last matmul, doing real work during the AR1 wait
            weight_prep(1)

            # corr[o] = sum over ib1 taps of wsign2[:, o]: 9 accumulating
            # N=1 matmuls against a ones vector, per output block; feeds
            # fold2's {0,2}-encoding fixup (all off the critical path).
            psc = pp.tile([128, 512], F32, name="cps", tag="cps")
            for ob in range(NOB):
                for t in range(NTAP):
                    nc.tensor.matmul(
                        psc[:, ob:ob + 1],
                        wsign[1][:, WS + (ob * NTAP + t) * 128:
                                 WS + (ob * NTAP + t + 1) * 128],
                        jw[:, 0:1],
                        start=(t == 0), stop=(t == NTAP - 1),
                    )
            with tc.high_priority(offset=-6000):
                nc.vector.tensor_copy(corrt[:, 0:2], psc[:, 0:2])
                nc.vector.tensor_scalar_mul(corrt[:, 2:4], corrt[:, 0:2], 0.5)
                nc.vector.tensor_scalar_mul(corrt[:, 4:6], corrt[:, 0:2], 4.0)
                nc.vector.tensor_mul(corrt[:, 6:8], corrt[:, 0:2],
                                     corrt[:, 0:2])
            # conv2's ib1 planes are {0,2}-encoded (= sign+1): their pad
            # ring must read as +1 so pad contributes sign 0 after the
            # correction. Rewritten here, in the AR1 window, after conv1's
            # last matmul has read the zero ring (tile WAR orders this).
            for i in range(2):
                blk = ab2[i][:, ABW:2 * ABW]
                nc.vector.memset(blk[:, 0:61], 1.0)
                nc.vector.memset(blk[:, 3307:ABW], 1.0)
                ring = blk[:, 117:117 + 55 * 58].rearrange(
                    "p (r c) -> p r c", c=58)[:, :, 0:2]
                nc.vector.memset(ring, 1.0)

            if phase_lim >= 2:
                # PE HAM bridge across the AR1b wait: free-running junk
                # matmuls (NOT gated on the fold) keep the clock-gate warm
                # from conv1's end until the fold lands; conv2's first real
                # matmul then queues behind at most the leftover junk.
                warm2 = pp.tile([128, 512], F32, name="cps", tag="cps")
                for i in range(NBRIDGE):
                    nc.tensor.matmul(warm2[:, 0:512], jw[:, 0:128], jw[:],
                                     start=(i == 0), stop=(i == NBRIDGE - 1))
                nc.vector.tensor_reduce(
                    out=btmp[1][:, 13:14], in_=warm2[:, 0:128],
                    axis=AX.X, op=ALU.add)

            # ---------------- conv2: acts = sign(s1*z1 + b1) ----------------
            def fill2(n, abuf):
                for ib in range(NIB):
                    a58 = abuf[:, ib * ABW + GB:ib * ABW + GB + SPP].rearrange(
                        "p (h w) -> p h w", w=HP)
                    zv = zstore[0][:, (n * NOB + ib) * SP:
                                   (n * NOB + ib + 1) * SP].rearrange(
                        "p (h w) -> p h w", w=W)
                    with tc.high_priority(offset=None if n == 0 else 0):
                        if ib == 0:
                            nc.scalar.activation(
                                out=a58[:, 1:H + 1, 1:W + 1], in_=zv,
                                func=ACTF.Sign,
                                scale=coef[0][:, ib:ib + 1],
                                bias=coef[0][:, NOB + ib:NOB + ib + 1],
                            )
                        else:
                            # DVE-emulated sign in {0,2} encoding (= sign+1,
                            # ONE op; fold2 corrects stats/coefs), so the two
                            # channel blocks fill in parallel on ACT and DVE:
                            # (z >= thr)*2 = (sign(s*z + b) + 1) for s > 0
                            av = a58[:, 1:H + 1, 1:W + 1]
                            nc.vector.tensor_scalar(
                                out=av, in0=zv,
                                scalar1=btmp[0][:, 14 + ib:15 + ib],
                                scalar2=2.0, op0=ALU.is_ge, op1=ALU.mult)

            if phase_lim >= 3:
                conv_pass(1, fill2, do_ar=(phase_lim >= 9))

            if phase_lim < 9:
                # debug: dump something touching live tiles into out
                dbg = pd.tile([128, SP], F16, name="dbg", tag="t16", bufs=6)
                if phase_lim >= 1:
                    nc.vector.tensor_copy(dbg[:], zstore[0][:, 0:SP])
                else:
                    nc.vector.tensor_copy(dbg[:], wsign[0][:, 0:SP])
                nc.sync.dma_start(out_ap[0, 0:128, :], dbg[:])

            # ---------------- finalize: out = s2*z2' + b2 + x ----------------
            # Per block: per-channel affine (alternating ACT / DVE so neither
            # engine is the serial bottleneck), DVE residual add from the
            # resident fp16 x, fp16 store alternating sync/scalar queues.
            for n in range(NPER if phase_lim >= 9 else 0):
                for ob in range(NOB):
                    k = n * NOB + ob
                    zsl = zstore[1][:, (n * NOB + ob) * SP:
                                    (n * NOB + ob + 1) * SP]
                    xsl = x16n[n][:, ob * SP:(ob + 1) * SP]
                    t16 = pd.tile([128, SP], F16, name="t16", tag="t16",
                                  bufs=6)
                    if k % 2 == 0:
                        nc.scalar.activation(
                            out=t16[:], in_=zsl, func=ACTF.Identity,
                            scale=coef[1][:, ob:ob + 1],
                            bias=coef[1][:, NOB + ob:NOB + ob + 1],
                        )
                    else:
                        nc.vector.tensor_scalar(
                            out=t16[:], in0=zsl,
                            scalar1=coef[1][:, ob:ob + 1],
                            scalar2=coef[1][:, NOB + ob:NOB + ob + 1],
                            op0=ALU.mult, op1=ALU.add,
                        )
                    # residual adds all on DVE (gpsimd bulk elementwise is
                    # ~4x slower AND its SBUF traffic stalls concurrent DVE)
                    nc.vector.tensor_add(t16[:], t16[:], xsl)
                    dma_eng = (nc.sync, nc.scalar)[k % 2]
                    dma_eng.dma_start(
                        out_ap[n, ob * 128:(ob + 1) * 128, :], t16[:])

    nc.compile()
    return nc


def _get_nc(num_devices=NCORES):
    if num_devices not in _nc_cache:
        _nc_cache[num_devices] = build_nc(num_devices)
    return _nc_cache[num_devices]


def make_in_maps(inputs):
    import ml_dtypes

    x = np.ascontiguousarray(
        np.asarray(inputs["x"], dtype=np.float32).astype(np.float16))
    shared = {
        "w1b": np.ascontiguousarray(
            np.asarray(inputs["w1"], dtype=np.float32)
            .reshape(C, KELEM).astype(ml_dtypes.bfloat16)),
        "w2b": np.ascontiguousarray(
            np.asarray(inputs["w2"], dtype=np.float32)
            .reshape(C, KELEM).astype(ml_dtypes.bfloat16)),
    }
    for k in ("gamma1", "beta1", "gamma2", "beta2"):
        shared[k] = np.ascontiguousarray(
            np.asarray(inputs[k], dtype=np.float32))
    return [
        {"x": x[c * NPER:(c + 1) * NPER], **shared} for c in range(NCORES)
    ]


def kernel(**inputs):
    from concourse.bass_utils import run_bass_kernel_spmd

    nc = _get_nc(NCORES)
    in_maps = make_in_maps(inputs)
    res = run_bass_kernel_spmd(nc, in_maps, core_ids=list(range(NCORES)))
    out = np.concatenate([np.asarray(r["out"]) for r in res.results], axis=0)
    return out.astype(np.float32)
